# revision 1
# baseline (speedup 1.0000x reference)
"""Trainium2 Bass kernel for nn_ClassicalSelfAttention (B=4, S=2048, E=1024).

Reference computation (fp32):
    w_qkv = rotation_params.reshape(3E, E); w_out = entangle_params.reshape(E, E)
    qkv = x @ w_qkv.T; q, k, v = split(qkv)
    scores = (q / sqrt(64)) @ k.T          # full-E attention, no heads
    attn = softmax(scores, axis=-1)
    out = (attn @ v) @ w_out.T
    result = sigmoid(out @ gate_w.T) * out

Sharding: 8 cores = 4 batches x 2 query-halves. Each core computes K/V for its
whole batch (duplicated within the pair) and attention + projections for its
1024 queries. Key order is rotated per query-half so each core's queries are
always columns 0:1024 of its (host-pre-transposed) x^T input — softmax and
attn@v are permutation-invariant in key order.

All heavy matmuls run in float32r (fp32 with 11-bit mantissa, full PE speed at
free-dim 512). Data layout is feature-major ("transposed") throughout:
    xT [e, s] -> qT [f, s], kT [f, s] (moving/stationary for scores)
               -> v [s, f] natural (stationary for attn@v)
    scores [qi, kj] -> softmax along free dim -> normalized attn
    PE-transpose attn -> attnT [kj, qi]
    attn_outT [e, qi] = v.T @ attnT
    outT [f, qi] = w_outT.T @ attn_outT
    gateT [f', qi] = gw.T.T @ outT;  result^T = sigmoid(gateT) * outT
Host untransposes the per-core [E, 1024] result tiles.
"""

from contextlib import ExitStack

import numpy as np

import concourse.bass as bass
import concourse.tile as tile
from concourse import bacc, mybir
from concourse.bass_utils import run_bass_kernel_spmd
from concourse.masks import make_identity

F32 = mybir.dt.float32
F32R = mybir.dt.float32r

P = 128
E = 1024
B = 4
S = 2048
SK = S            # keys per core (full batch sequence)
SQ = S // 2       # queries per core (half)
ET = E // P       # 8 e-tiles
KT = SK // P      # 16 key tiles
NC = 512          # moving-operand chunk (f32r full speed needs >=256, max 512)
SKC = SK // NC    # 4
SQC = SQ // NC    # 2
FC = E // NC      # 2
NCORES = 8
SCALE = 1.0 / 8.0  # 1/sqrt(head_dim=64), folded into exp()


def _round_fp32r(x: np.ndarray) -> np.ndarray:
    """Round-to-nearest-even to fp32r (11-bit mantissa; low 12 bits zero)."""
    u = np.ascontiguousarray(x, dtype=np.float32).view(np.uint32).astype(np.uint64)
    r = (u + 0x7FF + ((u >> 12) & 1)) & ~np.uint64(0xFFF)
    return r.astype(np.uint32).view(np.float32)


def _build_nc():
    nc = bacc.Bacc("TRN2", target_bir_lowering=False, debug=False,
                   num_devices=NCORES)
    xT = nc.dram_tensor("xT", [E, SK], F32R, kind="ExternalInput").ap()
    wqT = nc.dram_tensor("wqT", [E, E], F32R, kind="ExternalInput").ap()
    wkT = nc.dram_tensor("wkT", [E, E], F32R, kind="ExternalInput").ap()
    wvT = nc.dram_tensor("wvT", [E, E], F32R, kind="ExternalInput").ap()
    woT = nc.dram_tensor("woT", [E, E], F32R, kind="ExternalInput").ap()
    gwT = nc.dram_tensor("gwT", [E, E], F32R, kind="ExternalInput").ap()
    outT = nc.dram_tensor("outT", [E, SQ], F32, kind="ExternalOutput").ap()

    with tile.TileContext(nc) as tc, ExitStack() as ctx:
        _emit(tc, ctx, xT, wqT, wkT, wvT, woT, gwT, outT)
    nc.compile()
    return nc


def _emit(tc, ctx, xT, wqT, wkT, wvT, woT, gwT, outT):
    nc = tc.nc
    Exp = mybir.ActivationFunctionType.Exp
    Sigmoid = mybir.ActivationFunctionType.Sigmoid

    singles = ctx.enter_context(tc.tile_pool(name="singles", bufs=1))
    ident_f = singles.tile([P, P], F32, tag="ident_f")
    make_identity(nc, ident_f)
    ident = singles.tile([P, P], F32R, tag="ident")
    nc.vector.tensor_copy(out=ident[:], in_=ident_f[:])

    dram = ctx.enter_context(tc.tile_pool(name="dram", bufs=1, space="DRAM"))
    vsp = dram.tile([SK, E], F32R, tag="vsp")

    # Staged-lifetime SBUF pools (overlapping, hence explicit ExitStacks):
    #   qt/kt: phase1 -> end of 2a;  att: 2a -> end of 2b;  aot: 2b -> end.
    ps_mm = ctx.enter_context(tc.tile_pool(name="ps_mm", bufs=6, space="PSUM"))

    es_qk = ExitStack()
    qt_pool = es_qk.enter_context(tc.tile_pool(name="qt", bufs=1))
    kt_pool = es_qk.enter_context(tc.tile_pool(name="kt", bufs=1))
    qt = [qt_pool.tile([P, SQ], F32R, tag=f"qt{i}", name=f"qt{i}") for i in range(ET)]
    kt = [kt_pool.tile([P, SK], F32R, tag=f"kt{i}", name=f"kt{i}") for i in range(ET)]

    # ---------------- Phase 1: qT, v (spilled), kT ----------------
    with tc.tile_pool(name="xt", bufs=1) as xt_pool, \
         tc.tile_pool(name="wp", bufs=1) as w_pool, \
         tc.tile_pool(name="vb", bufs=2) as vb_pool:

        # interleaved loads: qT (first compute) needs only wq + xT cols 0:SQ
        xt, wq = [], []
        for et in range(ET):
            tw = w_pool.tile([P, E], F32R, tag=f"w{et}", name=f"wq{et}")
            nc.sync.dma_start(out=tw[:], in_=wqT[et * P:(et + 1) * P, :])
            wq.append(tw)
            t = xt_pool.tile([P, SK], F32R, tag=f"xt{et}", name=f"xt{et}")
            nc.sync.dma_start(out=t[:, 0:SQ], in_=xT[et * P:(et + 1) * P, 0:SQ])
            xt.append(t)
        # --- qT[f, s] = wqT.T @ xq
        for ft in range(ET):
            psums = [ps_mm.tile([P, NC], F32, tag="mm", name="mmp") for _ in range(SQC)]
            for et in range(ET):
                for sc in range(SQC):
                    nc.tensor.matmul(
                        psums[sc][:],
                        wq[et][:, ft * P:(ft + 1) * P],
                        xt[et][:, sc * NC:(sc + 1) * NC],
                        start=(et == 0), stop=(et == ET - 1),
                    )
            for sc in range(SQC):
                nc.vector.tensor_copy(
                    out=qt[ft][:, sc * NC:(sc + 1) * NC], in_=psums[sc][:])

        # --- v[s, f] = xT.T @ wvT : stationary xT block, moving wv; spill to DRAM
        wv = []
        for et in range(ET):
            t = w_pool.tile([P, E], F32R, tag=f"w{et}")
            nc.sync.dma_start(out=t[:], in_=wvT[et * P:(et + 1) * P, :])
            wv.append(t)
            nc.sync.dma_start(
                out=xt[et][:, SQ:SK], in_=xT[et * P:(et + 1) * P, SQ:SK])
        for st in range(KT):
            psums = [ps_mm.tile([P, NC], F32, tag="mm", name="mmp") for _ in range(FC)]
            for et in range(ET):
                for fc in range(FC):
                    nc.tensor.matmul(
                        psums[fc][:],
                        xt[et][:, st * P:(st + 1) * P],
                        wv[et][:, fc * NC:(fc + 1) * NC],
                        start=(et == 0), stop=(et == ET - 1),
                    )
            vb = vb_pool.tile([P, E], F32R, tag="vb")
            for fc in range(FC):
                nc.vector.tensor_copy(
                    out=vb[:, fc * NC:(fc + 1) * NC], in_=psums[fc][:])
            nc.sync.dma_start(out=vsp[st * P:(st + 1) * P, :], in_=vb[:])

        # --- kT[f, s] = wkT.T @ xT (full SK columns)
        wk = []
        for et in range(ET):
            t = w_pool.tile([P, E], F32R, tag=f"w{et}")
            nc.sync.dma_start(out=t[:], in_=wkT[et * P:(et + 1) * P, :])
            wk.append(t)
        for ft in range(ET):
            psums = [ps_mm.tile([P, NC], F32, tag="mm", name="mmp") for _ in range(SKC)]
            for et in range(ET):
                for kc in range(SKC):
                    nc.tensor.matmul(
                        psums[kc][:],
                        wk[et][:, ft * P:(ft + 1) * P],
                        xt[et][:, kc * NC:(kc + 1) * NC],
                        start=(et == 0), stop=(et == ET - 1),
                    )
            for kc in range(SKC):
                nc.vector.tensor_copy(
                    out=kt[ft][:, kc * NC:(kc + 1) * NC], in_=psums[kc][:])

    # ---------------- Phase 2a: scores -> softmax -> attnT ----------------
    es_att = ExitStack()
    att_pool = es_att.enter_context(tc.tile_pool(name="att", bufs=1, side="right"))
    att = [att_pool.tile([P, SQ], F32R, tag=f"at{i}", name=f"at{i}") for i in range(KT)]

    with tc.tile_pool(name="exp", bufs=2) as exp_pool, \
         tc.tile_pool(name="sums", bufs=4) as sums_pool, \
         tc.tile_pool(name="ps_t", bufs=2, space="PSUM") as ps_t:

        for sb in range(ET):  # 8 query sub-blocks of 128
            psums = [ps_mm.tile([P, NC], F32, tag="mm", name="mmp") for _ in range(SKC)]
            for et in range(ET):
                for kc in range(SKC):
                    nc.tensor.matmul(
                        psums[kc][:],
                        qt[et][:, sb * P:(sb + 1) * P],
                        kt[et][:, kc * NC:(kc + 1) * NC],
                        start=(et == 0), stop=(et == ET - 1),
                    )
            exp_t = exp_pool.tile([P, SK], F32, tag="exp")
            sums4 = sums_pool.tile([P, SKC], F32, tag="sums4")
            for kc in range(SKC):
                nc.scalar.activation(
                    out=exp_t[:, kc * NC:(kc + 1) * NC],
                    in_=psums[kc][:], func=Exp, scale=SCALE,
                    accum_out=sums4[:, kc:kc + 1],
                )
            sum1 = sums_pool.tile([P, 1], F32, tag="sum1")
            nc.vector.tensor_reduce(
                out=sum1[:], in_=sums4[:],
                axis=mybir.AxisListType.X, op=mybir.AluOpType.add)
            recip = sums_pool.tile([P, 1], F32, tag="recip")
            nc.vector.reciprocal(out=recip[:], in_=sum1[:])
            attn_n = exp_pool.tile([P, SK], F32R, tag="attn_n", bufs=2)
            nc.scalar.mul(out=attn_n[:], in_=exp_t[:], mul=recip[:])
            for kj in range(KT):
                pst = ps_t.tile([P, P], F32R, tag="pst")
                nc.tensor.transpose(
                    pst[:], attn_n[:, kj * P:(kj + 1) * P], ident[:])
                nc.vector.tensor_copy(
                    out=att[kj][:, sb * P:(sb + 1) * P], in_=pst[:])

    # ---------------- Phase 2b: attn_outT[e, qi] = v.T @ attnT ----------------
    es_qk.close()  # qt/kt freed after scores
    aot_pool = ctx.enter_context(tc.tile_pool(name="aot", bufs=1))
    aot = [aot_pool.tile([P, SQ], F32R, tag=f"ao{i}", name=f"ao{i}") for i in range(ET)]

    w2_es = ExitStack()
    w2_pool = w2_es.enter_context(tc.tile_pool(name="wp2", bufs=1))
    with tc.tile_pool(name="vt", bufs=1) as v_pool:
        vt = []
        for st in range(KT):
            t = v_pool.tile([P, E], F32R, tag=f"v{st}", name=f"v{st}")
            nc.sync.dma_start(out=t[:], in_=vsp[st * P:(st + 1) * P, :])
            vt.append(t)
        # prefetch out-projection weights during attn@v
        wo = []
        for et in range(ET):
            t = w2_pool.tile([P, E], F32R, tag=f"w2{et}", name=f"wo{et}")
            nc.sync.dma_start(out=t[:], in_=woT[et * P:(et + 1) * P, :])
            wo.append(t)
        for et in range(ET):
            psums = [ps_mm.tile([P, NC], F32, tag="mm", name="mmp") for _ in range(SQC)]
            for kj in range(KT):
                for qc in range(SQC):
                    nc.tensor.matmul(
                        psums[qc][:],
                        vt[kj][:, et * P:(et + 1) * P],
                        att[kj][:, qc * NC:(qc + 1) * NC],
                        start=(kj == 0), stop=(kj == KT - 1),
                    )
            for qc in range(SQC):
                nc.vector.tensor_copy(
                    out=aot[et][:, qc * NC:(qc + 1) * NC], in_=psums[qc][:])

    es_att.close()  # att freed after attn@v

    # ---------------- Phase 2c: outT, gate, result ----------------
    with tc.tile_pool(name="ot", bufs=1) as ot_pool, \
         tc.tile_pool(name="fin", bufs=2) as fin_pool:

        ot = [ot_pool.tile([P, SQ], F32R, tag=f"ot{i}", name=f"ot{i}") for i in range(ET)]
        for ft in range(ET):
            psums = [ps_mm.tile([P, NC], F32, tag="mm", name="mmp") for _ in range(SQC)]
            for et in range(ET):
                for qc in range(SQC):
                    nc.tensor.matmul(
                        psums[qc][:],
                        wo[et][:, ft * P:(ft + 1) * P],
                        aot[et][:, qc * NC:(qc + 1) * NC],
                        start=(et == 0), stop=(et == ET - 1),
                    )
            for qc in range(SQC):
                nc.vector.tensor_copy(
                    out=ot[ft][:, qc * NC:(qc + 1) * NC], in_=psums[qc][:])

        gw = []
        for et in range(ET):
            t = w2_pool.tile([P, E], F32R, tag=f"w2{et}")
            nc.sync.dma_start(out=t[:], in_=gwT[et * P:(et + 1) * P, :])
            gw.append(t)
        for ft in range(ET):
            psums = [ps_mm.tile([P, NC], F32, tag="mm", name="mmp") for _ in range(SQC)]
            for et in range(ET):
                for qc in range(SQC):
                    nc.tensor.matmul(
                        psums[qc][:],
                        gw[et][:, ft * P:(ft + 1) * P],
                        ot[et][:, qc * NC:(qc + 1) * NC],
                        start=(et == 0), stop=(et == ET - 1),
                    )
            fin = fin_pool.tile([P, SQ], F32, tag="fin")
            for qc in range(SQC):
                gate = fin_pool.tile([P, NC], F32, tag="gate")
                nc.scalar.activation(
                    out=gate[:], in_=psums[qc][:], func=Sigmoid)
                nc.vector.tensor_mul(
                    fin[:, qc * NC:(qc + 1) * NC], gate[:],
                    ot[ft][:, qc * NC:(qc + 1) * NC].bitcast(F32))
            nc.sync.dma_start(out=outT[ft * P:(ft + 1) * P, :], in_=fin[:])

    w2_es.close()


_NC_CACHE = None


def _get_nc():
    global _NC_CACHE
    if _NC_CACHE is None:
        _NC_CACHE = _build_nc()
    return _NC_CACHE


def _prep_in_maps(rotation_params, entangle_params, inputs, gate_w):
    w_qkv = np.asarray(rotation_params, dtype=np.float32).reshape(3 * E, E)
    wq, wk, wv = w_qkv[:E], w_qkv[E:2 * E], w_qkv[2 * E:]
    w_out = np.asarray(entangle_params, dtype=np.float32).reshape(E, E)
    gw = np.asarray(gate_w, dtype=np.float32)
    x = np.asarray(inputs, dtype=np.float32)

    wqT = _round_fp32r(wq.T)
    wkT = _round_fp32r(wk.T)
    wvT = _round_fp32r(wv.T)
    woT = _round_fp32r(w_out.T)
    gwT = _round_fp32r(gw.T)

    in_maps = []
    for c in range(NCORES):
        b, h = c // 2, c % 2
        xT = x[b].T  # [E, S]
        if h == 1:   # rotate keys so this core's queries sit at columns 0:SQ
            xT = np.concatenate([xT[:, SQ:], xT[:, :SQ]], axis=1)
        in_maps.append({
            "xT": _round_fp32r(xT),
            "wqT": wqT, "wkT": wkT, "wvT": wvT, "woT": woT, "gwT": gwT,
        })
    return in_maps


def _assemble(results):
    out = np.empty((B, S, E), dtype=np.float32)
    for c in range(NCORES):
        b, h = c // 2, c % 2
        out[b, h * SQ:(h + 1) * SQ, :] = results[c]["outT"].T
    return out


def _run(in_maps, trace=False):
    nc = _get_nc()
    return run_bass_kernel_spmd(nc, in_maps, core_ids=list(range(NCORES)),
                                trace=trace)


def kernel(rotation_params, entangle_params, inputs, gate_w):
    in_maps = _prep_in_maps(rotation_params, entangle_params, inputs, gate_w)
    res = _run(in_maps, trace=False)
    return _assemble(res.results)



# revision 6
# speedup vs baseline: 1.7869x; 1.7869x over previous
"""Trainium2 Bass kernel for nn_ClassicalSelfAttention (B=4, S=2048, E=1024).

Reference computation (fp32):
    w_qkv = rotation_params.reshape(3E, E); w_out = entangle_params.reshape(E, E)
    qkv = x @ w_qkv.T; q, k, v = split(qkv)
    scores = (q / sqrt(64)) @ k.T          # full-E attention, no heads
    attn = softmax(scores, axis=-1)
    out = (attn @ v) @ w_out.T
    result = sigmoid(out @ gate_w.T) * out

Algebraic folding (host-side, fp32):
    M  = Wq^T Wk / 8     -> scores = (x M) x^T       (kills the K projection)
    W2 = wo Wv           -> out    = (attn x) W2^T   (kills the V projection)
    W3 = gw wo Wv        -> gate_l = (attn x) W3^T   (decouples gate from out)

Sharding: 8 cores = 4 batches x 2 query-halves. Key order is rotated per
query-half so each core's queries are always columns 0:1024 of its xT input
(softmax and attn@x are permutation-invariant in key order).

All matmuls run in bf16 (full PE speed), fp32 PSUM accumulation. Softmax is
computed transposed (scoresT[kj, qi]) so no PE transpose of attn is needed:
    q'T[f, qi]    = M.T-blocks @ xT            (q' projection)
    scoresT[kj, qi] = xT[:,kj-block].T @ q'T   (stationary x, moving q')
    expT = exp(scoresT)     unnormalized, bf16 (scalar engine, psum -> sbuf)
    denom[1, qi] = ones[128,1].T @ expT        (PE, accumulated over kj tiles)
    ao_unT[e, qi] = xn[kj,e-block].T @ expT    (stationary xn, moving expT)
    rb[128, qi] = ones_row.T @ (1/denom)       (PE broadcast of reciprocal)
    aoT = ao_unT * rb       (normalization folded into psum->sbuf copy, DVE)
    outT = W2T-blocks @ aoT;  gateT = W3T-blocks @ aoT
    result^T = sigmoid(gateT) * outT           (bf16 out, DMA per f-tile)
Host untransposes the per-core [E, 1024] bf16 result tiles.
"""

from contextlib import ExitStack

import numpy as np
import ml_dtypes

import concourse.bass as bass
import concourse.tile as tile
from concourse import bacc, mybir
from concourse.bass_utils import run_bass_kernel_spmd

F32 = mybir.dt.float32
BF16 = mybir.dt.bfloat16
NPBF16 = ml_dtypes.bfloat16

P = 128
E = 1024
B = 4
S = 2048
SK = S            # keys per core (full batch sequence)
SQ = S // 2       # queries per core (half)
ET = E // P       # 8 e-tiles
KT = SK // P      # 16 key tiles
NC = 512          # moving-operand chunk
QC = SQ // NC     # 2 query chunks
NCORES = 8


def _build_nc():
    nc = bacc.Bacc("TRN2", target_bir_lowering=False, debug=False,
                   num_devices=NCORES)
    xT = nc.dram_tensor("xT", [E, SK], BF16, kind="ExternalInput").ap()
    xn = nc.dram_tensor("xn", [SK, E], BF16, kind="ExternalInput").ap()
    m = nc.dram_tensor("m", [E, E], BF16, kind="ExternalInput").ap()
    w2T = nc.dram_tensor("w2T", [E, E], BF16, kind="ExternalInput").ap()
    w3T = nc.dram_tensor("w3T", [E, E], BF16, kind="ExternalInput").ap()
    outT = nc.dram_tensor("outT", [E, SQ], BF16, kind="ExternalOutput").ap()

    with tile.TileContext(nc) as tc, ExitStack() as ctx:
        _emit(tc, ctx, xT, xn, m, w2T, w3T, outT)
    nc.compile()
    return nc


def _emit(tc, ctx, xT, xn, m, w2T, w3T, outT):
    nc = tc.nc
    Exp = mybir.ActivationFunctionType.Exp
    Sigmoid = mybir.ActivationFunctionType.Sigmoid

    singles = ctx.enter_context(tc.tile_pool(name="singles", bufs=1))
    ones_col = singles.tile([P, 1], BF16, tag="ones_col")
    nc.vector.memset(ones_col[:], 1.0)
    ones_row = singles.tile([1, P], F32, tag="ones_row")
    nc.vector.memset(ones_row[:], 1.0)

    ps_mm = ctx.enter_context(tc.tile_pool(name="ps_mm", bufs=6, space="PSUM"))
    ps_d = ctx.enter_context(tc.tile_pool(name="ps_d", bufs=1, space="PSUM"))

    # long-lived inputs
    xt_pool = ctx.enter_context(tc.tile_pool(name="xt", bufs=1))
    xn_pool = ctx.enter_context(tc.tile_pool(name="xn", bufs=1))
    w_pool = ctx.enter_context(tc.tile_pool(name="wp", bufs=1))

    # staged lifetimes
    es_m = ExitStack()
    m_pool = es_m.enter_context(tc.tile_pool(name="mp", bufs=1))
    es_q = ExitStack()
    qt_pool = es_q.enter_context(tc.tile_pool(name="qt", bufs=1))
    es_exp = ExitStack()
    exp_pool = es_exp.enter_context(tc.tile_pool(name="exp", bufs=1, side="right"))

    # ---------------- DMA: phase-1 critical loads ----------------
    mt, xt = [], []
    for et in range(ET):
        tm = m_pool.tile([P, E], BF16, tag=f"m{et}", name=f"m{et}")
        nc.sync.dma_start(out=tm[:], in_=m[et * P:(et + 1) * P, :])
        mt.append(tm)
        t = xt_pool.tile([P, SK], BF16, tag=f"xt{et}", name=f"xt{et}")
        nc.sync.dma_start(out=t[:, 0:SQ], in_=xT[et * P:(et + 1) * P, 0:SQ])
        xt.append(t)

    # ---------------- Phase 1: q'T[f, qi] = M-blocks.T @ xT[:, 0:SQ] --------
    qt = [qt_pool.tile([P, SQ], BF16, tag=f"q{i}", name=f"q{i}") for i in range(ET)]
    for ft in range(ET):
        psums = [ps_mm.tile([P, NC], F32, tag="mm", name="mmp") for _ in range(QC)]
        for et in range(ET):
            for qc in range(QC):
                nc.tensor.matmul(
                    psums[qc][:],
                    mt[et][:, ft * P:(ft + 1) * P],
                    xt[et][:, qc * NC:(qc + 1) * NC],
                    start=(et == 0), stop=(et == ET - 1),
                )
        for qc in range(QC):
            nc.vector.tensor_copy(
                out=qt[ft][:, qc * NC:(qc + 1) * NC], in_=psums[qc][:])

    # ---------------- DMA: rest of the inputs (overlaps phases 1-3) --------
    for et in range(ET):
        nc.sync.dma_start(
            out=xt[et][:, SQ:SK], in_=xT[et * P:(et + 1) * P, SQ:SK])
    xnt = []
    for kt in range(KT):
        t = xn_pool.tile([P, E], BF16, tag=f"xn{kt}", name=f"xn{kt}")
        nc.sync.dma_start(out=t[:], in_=xn[kt * P:(kt + 1) * P, :])
        xnt.append(t)
    w2t, w3t = [], []
    for et in range(ET):
        t = w_pool.tile([P, E], BF16, tag=f"w2{et}", name=f"w2{et}")
        nc.sync.dma_start(out=t[:], in_=w2T[et * P:(et + 1) * P, :])
        w2t.append(t)
    for et in range(ET):
        t = w_pool.tile([P, E], BF16, tag=f"w3{et}", name=f"w3{et}")
        nc.sync.dma_start(out=t[:], in_=w3T[et * P:(et + 1) * P, :])
        w3t.append(t)

    # ---------------- Phase 2: scoresT -> exp -> denom ----------------
    # scoresT[kj, qi] = sum_e xT[e, kj] q'T[e, qi]; denom accumulated on PE
    # via ones-matmuls with a one-tile emission lag (exp of tile kt runs on
    # the scalar engine while PE does scores of tile kt+1).
    ext = [exp_pool.tile([P, SQ], BF16, tag=f"ex{i}", name=f"ex{i}")
           for i in range(KT)]
    psd = [ps_d.tile([1, NC], F32, tag=f"d{qc}", name=f"d{qc}")
           for qc in range(QC)]

    def emit_denom(kt):
        for qc in range(QC):
            nc.tensor.matmul(
                psd[qc][:],
                ones_col[:],
                ext[kt][:, qc * NC:(qc + 1) * NC],
                start=(kt == 0), stop=(kt == KT - 1),
            )

    for kt in range(KT):
        psums = [ps_mm.tile([P, NC], F32, tag="mm", name="mmp") for _ in range(QC)]
        for et in range(ET):
            for qc in range(QC):
                nc.tensor.matmul(
                    psums[qc][:],
                    xt[et][:, kt * P:(kt + 1) * P],
                    qt[et][:, qc * NC:(qc + 1) * NC],
                    start=(et == 0), stop=(et == ET - 1),
                )
        for qc in range(QC):
            nc.scalar.activation(
                out=ext[kt][:, qc * NC:(qc + 1) * NC],
                in_=psums[qc][:], func=Exp,
            )
        if kt > 0:
            emit_denom(kt - 1)
    emit_denom(KT - 1)

    es_q.close()   # qt freed
    es_m.close()   # M freed

    # ---------------- Phase 3: ao_unT -> normalized aoT ----------------
    ao_pool = ctx.enter_context(tc.tile_pool(name="ao", bufs=1))
    rb_pool = ctx.enter_context(tc.tile_pool(name="rb", bufs=1))
    aot = [ao_pool.tile([P, SQ], BF16, tag=f"ao{i}", name=f"ao{i}")
           for i in range(ET)]
    recip = rb_pool.tile([1, SQ], F32, tag="recip")
    rb = rb_pool.tile([P, SQ], F32, tag="rb")

    for et in range(ET):
        psums = [ps_mm.tile([P, NC], F32, tag="mm", name="mmp") for _ in range(QC)]
        for kt in range(KT):
            for qc in range(QC):
                nc.tensor.matmul(
                    psums[qc][:],
                    xnt[kt][:, et * P:(et + 1) * P],
                    ext[kt][:, qc * NC:(qc + 1) * NC],
                    start=(kt == 0), stop=(kt == KT - 1),
                )
        if et == 0:
            # reciprocal + PE broadcast of 1/denom to all 128 partitions;
            # runs on DVE/PE while the et=1 accumulation is in flight.
            for qc in range(QC):
                nc.vector.reciprocal(
                    out=recip[:, qc * NC:(qc + 1) * NC], in_=psd[qc][:])
            psb = [ps_mm.tile([P, NC], F32, tag="mm", name="mmp")
                   for _ in range(QC)]
            for qc in range(QC):
                nc.tensor.matmul(
                    psb[qc][:],
                    ones_row[:],
                    recip[:, qc * NC:(qc + 1) * NC],
                    start=True, stop=True,
                )
                nc.vector.tensor_copy(
                    out=rb[:, qc * NC:(qc + 1) * NC], in_=psb[qc][:])
        for qc in range(QC):
            nc.vector.tensor_mul(
                aot[et][:, qc * NC:(qc + 1) * NC],
                psums[qc][:],
                rb[:, qc * NC:(qc + 1) * NC],
            )

    es_exp.close()  # expT freed

    # ---------------- Phase 4: outT, gateT, result ----------------
    fin_pool = ctx.enter_context(tc.tile_pool(name="fin", bufs=2))
    for ft in range(ET):
        ps_o = [ps_mm.tile([P, NC], F32, tag="mm", name="mmp") for _ in range(QC)]
        for et in range(ET):
            for qc in range(QC):
                nc.tensor.matmul(
                    ps_o[qc][:],
                    w2t[et][:, ft * P:(ft + 1) * P],
                    aot[et][:, qc * NC:(qc + 1) * NC],
                    start=(et == 0), stop=(et == ET - 1),
                )
        ps_g = [ps_mm.tile([P, NC], F32, tag="mm", name="mmp") for _ in range(QC)]
        for et in range(ET):
            for qc in range(QC):
                nc.tensor.matmul(
                    ps_g[qc][:],
                    w3t[et][:, ft * P:(ft + 1) * P],
                    aot[et][:, qc * NC:(qc + 1) * NC],
                    start=(et == 0), stop=(et == ET - 1),
                )
        fin = fin_pool.tile([P, SQ], BF16, tag="fin")
        for qc in range(QC):
            sg = fin_pool.tile([P, NC], BF16, tag="sg")
            nc.scalar.activation(out=sg[:], in_=ps_g[qc][:], func=Sigmoid)
            nc.vector.tensor_mul(
                fin[:, qc * NC:(qc + 1) * NC], ps_o[qc][:], sg[:])
        nc.sync.dma_start(out=outT[ft * P:(ft + 1) * P, :], in_=fin[:])


_NC_CACHE = None


def _get_nc():
    global _NC_CACHE
    if _NC_CACHE is None:
        _NC_CACHE = _build_nc()
    return _NC_CACHE


def _prep_in_maps(rotation_params, entangle_params, inputs, gate_w):
    w_qkv = np.asarray(rotation_params, dtype=np.float32).reshape(3 * E, E)
    wq, wk, wv = w_qkv[:E], w_qkv[E:2 * E], w_qkv[2 * E:]
    wo = np.asarray(entangle_params, dtype=np.float32).reshape(E, E)
    gw = np.asarray(gate_w, dtype=np.float32)
    x = np.asarray(inputs, dtype=np.float32)

    # host-folded weights (fp32 accuracy, then bf16)
    m = ((wq.T @ wk) / 8.0).astype(NPBF16)          # [e, f]
    w2 = wo @ wv                                     # [f_out, e]
    w2T = np.ascontiguousarray(w2.T).astype(NPBF16)  # [e, f]
    w3T = np.ascontiguousarray((gw @ w2).T).astype(NPBF16)

    in_maps = []
    for c in range(NCORES):
        b, h = c // 2, c % 2
        xb = x[b]
        if h == 1:   # rotate keys so this core's queries sit at rows 0:SQ
            xb = np.concatenate([xb[SQ:], xb[:SQ]], axis=0)
        in_maps.append({
            "xT": np.ascontiguousarray(xb.T).astype(NPBF16),
            "xn": xb.astype(NPBF16),
            "m": m, "w2T": w2T, "w3T": w3T,
        })
    return in_maps


def _assemble(results):
    out = np.empty((B, S, E), dtype=np.float32)
    for c in range(NCORES):
        b, h = c // 2, c % 2
        out[b, h * SQ:(h + 1) * SQ, :] = results[c]["outT"].astype(np.float32).T
    return out


def _run(in_maps, trace=False):
    nc = _get_nc()
    return run_bass_kernel_spmd(nc, in_maps, core_ids=list(range(NCORES)),
                                trace=trace)


def kernel(rotation_params, entangle_params, inputs, gate_w):
    in_maps = _prep_in_maps(rotation_params, entangle_params, inputs, gate_w)
    res = _run(in_maps, trace=False)
    return _assemble(res.results)
